# revision 2
# baseline (speedup 1.0000x reference)
"""Trainium2 Bass kernel for CircleProjectionLayer (ball projection, r=1).

out = center + d * min(1, 1/||d||),  d = x - center,  shapes [8388608, 3] f32.

Strategy vs the f32 baseline: the harness tolerance is rel_err < 2e-2 against
an output scale of ~4.6, while an end-to-end fp16 pipeline measures ~8e-4
relative error — so the whole kernel runs in fp16.  That halves HBM traffic
(the DMA roofline, this problem's target regime: 18 MiB/core vs 36) AND
doubles DVE throughput (fp16 tensor_tensor runs in 2x perf mode).

Sharding: pure data parallel — batch split 8 ways, one shard per NeuronCore.
Host casts f32 -> fp16 before upload and fp16 -> f32 after download (both
value-preserving to fp16 precision; the cast back is exact).

Per-core layout: the [1048576, 3] fp16 shard viewed flat as [128, 24576], so
each SBUF partition holds 8192 complete (x,y,z) rows contiguously; chunks of
W fp16 elements per partition stream through SBUF.

Engine split (all fp16), per chunk of R = W/3 rows:
  DVE   : d = x-c (dense, 2x); t = sq0+sq1 (dense planes, 2x);
          ss = max(t,eps)+sq2 (fused stt); m_k = d_k * scale (x3, strided, 1x);
          head fraction of out = m + c (dense, 2x) — rebalance knob f_split
  ACT   : sq = Square(d) written PLANAR (one op, strided out) so the row sums
          read dense; scale = Exp(-0.5*Relu(Ln(ss))) == min(1, rsqrt(ss)) with
          an exact clamp at 1 (one pre-placed table load covers all four:
          natural_log_exp_and_others; ACT Rsqrt is blocked in bass)
  GPSIMD: tail fraction of out = m + c (Q7 two-input floor ~2.5 cyc/elem)
  DMA   : x-in + center-in on the SP HWDGE ring; out on the Pool/SWDGE ring,
          queued right after the gpsimd add so its semaphore wait is already
          satisfied (no head-of-line blocking on either ring).
"""

import sys

sys.path.insert(0, "/opt/trn_rl_repo")

from contextlib import ExitStack

import numpy as np

import concourse.bass as bass
import concourse.tile as tile
from concourse import bacc, mybir
from concourse.bass_utils import run_bass_kernel_spmd
from concourse.hw_specs import get_activation_tables

F16 = mybir.dt.float16
AF = mybir.ActivationFunctionType
ALU = mybir.AluOpType

B = 8388608
N_CORES = 8
B_CORE = B // N_CORES          # 1048576 rows per core
P = 128
FPP = B_CORE * 3 // P          # 24576 fp16 elements per partition

IN_DTYPE = np.float16

_EPS = 6.1e-5                  # smallest normal fp16; keeps Ln's input sane
_ACT_SET = "natural_log_exp_and_others"


def _preload_act_table(nc):
    """Pre-place one LoadActFuncSet for the set containing Square/Ln/Relu/Exp
    so Bacc.insert_act_table_loads doesn't thrash between greedy choices."""
    tables = list(get_activation_tables(nc.m.arch).keys())
    set_id = tables.index(_ACT_SET)
    inst = mybir.InstLoadActFuncSet(
        name=nc.get_next_instruction_name(), act_func_set_id=set_id, ins=[], outs=[]
    )
    return nc.scalar.add_instruction(inst)


def _build(W=6144, bufs=3, schedule=None, loop_reps=1, f_split=0.85,
           planar_sq=True, preload_act=True):
    """`schedule`: optional explicit list of chunk widths (fp16 elems per
    partition, each a multiple of 6, summing to FPP). Default uniform W.
    `f_split`: fraction of the final out = m + c add done on GPSIMD (the
    rest runs on DVE, which has slack). `loop_reps`: wrap the schedule in a
    hardware For_i loop (benchmarking steady-state HW time only)."""
    if schedule is None:
        assert W % 6 == 0 and FPP % W == 0
        schedule = [W] * (FPP // W)
    assert sum(schedule) == FPP and all(w % 6 == 0 for w in schedule)
    W = max(schedule)

    nc = bacc.Bacc("TRN2", target_bir_lowering=False, debug=False)

    x = nc.dram_tensor("x", [B_CORE, 3], F16, kind="ExternalInput")
    c = nc.dram_tensor("center", [B_CORE, 3], F16, kind="ExternalInput")
    o = nc.dram_tensor("out", [B_CORE, 3], F16, kind="ExternalOutput")

    xr = x.ap().rearrange("(p f) c -> p (f c)", p=P)
    cr = c.ap().rearrange("(p f) c -> p (f c)", p=P)
    orr = o.ap().rearrange("(p f) c -> p (f c)", p=P)

    with tile.TileContext(nc) as tc, ExitStack() as ctx:
        if preload_act:
            _preload_act_table(nc)

        xp = ctx.enter_context(tc.tile_pool(name="xp", bufs=bufs))
        cp = ctx.enter_context(tc.tile_pool(name="cp", bufs=bufs))
        dp = ctx.enter_context(tc.tile_pool(name="dp", bufs=bufs))
        sqp = ctx.enter_context(tc.tile_pool(name="sqp", bufs=bufs))
        sp = ctx.enter_context(tc.tile_pool(name="sp", bufs=bufs))

        import contextlib
        loop_cm = tc.For_i(0, loop_reps, 1) if loop_reps > 1 else contextlib.nullcontext()
        with loop_cm:
            _emit_body(nc, schedule, W, xp, cp, dp, sqp, sp, xr, cr, orr,
                       f_split=f_split, planar_sq=planar_sq)

    nc.compile()
    return nc


def _emit_body(nc, schedule, W, xp, cp, dp, sqp, sp, xr, cr, orr,
               f_split=0.85, planar_sq=True):
    off = 0
    for i, w in enumerate(schedule):
        r = w // 3
        xt = xp.tile([P, W], F16, name="xt", tag="xt")[:, :w]
        nc.sync.dma_start(xt[:, :], xr[:, off : off + w])
        ct = cp.tile([P, W], F16, name="ct", tag="ct")[:, :w]
        nc.sync.dma_start(ct[:, :], cr[:, off : off + w])

        dt = dp.tile([P, W], F16, name="dt", tag="dt")[:, :w]
        nc.vector.tensor_sub(dt[:, :], xt[:, :], ct[:, :])

        # Squares, written as three dense planes [3, r] so row sums read dense.
        sq = sqp.tile([P, W], F16, name="sq", tag="sq")[:, :w]
        d3 = dt.rearrange("p (r c) -> p r c", c=3)
        if planar_sq:
            # One ACT op: in dense-interleaved, out strided planar.
            sq_pl = sq.rearrange("p (c r) -> p r c", c=3)
            nc.scalar.activation(sq_pl[:, :, :], d3[:, :, :], AF.Square)
        else:
            for k in range(3):
                nc.scalar.activation(sq[:, k * r : (k + 1) * r], d3[:, :, k],
                                     AF.Square)

        ta = sp.tile([P, W // 3], F16, name="ta", tag="ta")[:, :r]
        nc.vector.tensor_add(ta[:, :], sq[:, 0:r], sq[:, r : 2 * r])
        tb = sp.tile([P, W // 3], F16, name="tb", tag="tb")[:, :r]
        nc.vector.scalar_tensor_tensor(
            tb[:, :], ta[:, :], _EPS, sq[:, 2 * r : 3 * r], ALU.max, ALU.add
        )
        # scale = exp(-0.5*relu(ln(ss))) == min(1, rsqrt(ss)), exact clamp.
        nc.scalar.activation(ta[:, :], tb[:, :], AF.Ln)
        nc.scalar.activation(tb[:, :], ta[:, :], AF.Relu)
        sc = sp.tile([P, W // 3], F16, name="sc", tag="sc")[:, :r]
        nc.scalar.activation(sc[:, :], tb[:, :], AF.Exp, scale=-0.5)

        # m_k = d_k * scale, into the sq tile (squares are dead now).
        m3 = sq.rearrange("p (r c) -> p r c", c=3)
        for k in range(3):
            nc.vector.tensor_mul(m3[:, :, k], d3[:, :, k], sc[:, :])

        # out = m + c, into the d tile (d is dead now).  DVE head / GPSIMD
        # tail; the out-DMA right after the gpsimd add keeps the SWDGE queue
        # free of head-of-line waits.
        w1 = int(w * (1.0 - f_split)) // 6 * 6
        if w1 > 0:
            nc.vector.tensor_add(dt[:, :w1], sq[:, :w1], ct[:, :w1])
        if w1 < w:
            nc.gpsimd.tensor_add(dt[:, w1:], sq[:, w1:], ct[:, w1:])

        nc.gpsimd.dma_start(orr[:, off : off + w], dt[:, :])
        off += w


_NC = None

# Uniform 6144-wide chunks with a 2x3072 taper: the taper shortens the
# pipeline tail (last chunk's compute chain + out-DMA trail the stream).
_SCHEDULE = [6144] * 3 + [3072] * 2


def _get_nc():
    global _NC
    if _NC is None:
        _NC = _build(schedule=_SCHEDULE)
    return _NC


def kernel(**inputs):
    x = np.asarray(inputs["x"], dtype=np.float32)
    center = np.asarray(inputs["center"], dtype=np.float32)
    assert x.shape == (B, 3) and center.shape == (B, 3)

    x16 = x.astype(np.float16)
    c16 = center.astype(np.float16)
    xs = x16.reshape(N_CORES, B_CORE, 3)
    cs = c16.reshape(N_CORES, B_CORE, 3)
    in_maps = [
        {"x": np.ascontiguousarray(xs[i]), "center": np.ascontiguousarray(cs[i])}
        for i in range(N_CORES)
    ]

    nc = _get_nc()
    res = run_bass_kernel_spmd(nc, in_maps, list(range(N_CORES)))
    out = np.concatenate([res.results[i]["out"] for i in range(N_CORES)], axis=0)
    return out.astype(np.float32)


if __name__ == "__main__":
    nc = _get_nc()
    print("build ok")


# revision 9
# speedup vs baseline: 1.0984x; 1.0984x over previous
"""Trainium2 Bass kernel for CircleProjectionLayer (ball projection, r=1).

out = center + d * min(1, 1/||d||),  d = x - center,  shapes [8388608, 3] f32.

Strategy vs the f32 baseline: the harness tolerance is rel_err < 2e-2 against
an output scale of ~4.6, while an end-to-end fp16 pipeline measures ~8e-4
relative error — so the whole kernel runs in fp16.  That halves HBM traffic
(the DMA roofline, this problem's target regime: 18 MiB/core vs 36) AND
doubles DVE throughput (fp16 tensor_tensor runs in 2x perf mode).

Sharding: pure data parallel — batch split 8 ways, one shard per NeuronCore.
Host casts f32 -> fp16 before upload and fp16 -> f32 after download (both
value-preserving to fp16 precision; the cast back is exact).

Per-core layout: the [1048576, 3] fp16 shard viewed flat as [128, 24576], so
each SBUF partition holds 8192 complete (x,y,z) rows contiguously; chunks of
W fp16 elements per partition stream through SBUF.

Engine split (all fp16), per chunk of R = W/3 rows:
  DVE   : d = x-c (dense, 2x); t = sq0+sq1 (dense planes, 2x);
          ss = max(t,eps)+sq2 (fused stt); m_k = d_k * scale (x3, strided, 1x);
          head fraction of out = m + c (dense, 2x) — rebalance knob f_split
  ACT   : sq = Square(d) written PLANAR (one op, strided out) so the row sums
          read dense; scale = Exp(-0.5*Relu(Ln(ss))) == min(1, rsqrt(ss)) with
          an exact clamp at 1 (one pre-placed table load covers all four:
          natural_log_exp_and_others; ACT Rsqrt is blocked in bass)
  GPSIMD: tail fraction of out = m + c (Q7 two-input floor ~2.5 cyc/elem)
  DMA   : x-in + center-in on the SP HWDGE ring; out on the Pool/SWDGE ring,
          queued right after the gpsimd add so its semaphore wait is already
          satisfied (no head-of-line blocking on either ring).
"""

import sys

sys.path.insert(0, "/opt/trn_rl_repo")

from contextlib import ExitStack

import numpy as np

import concourse.bass as bass
import concourse.tile as tile
from concourse import bacc, mybir
from concourse.bass_utils import run_bass_kernel_spmd
from concourse.hw_specs import get_activation_tables

F16 = mybir.dt.float16
AF = mybir.ActivationFunctionType
ALU = mybir.AluOpType

B = 8388608
N_CORES = 8
B_CORE = B // N_CORES          # 1048576 rows per core
P = 128
FPP = B_CORE * 3 // P          # 24576 fp16 elements per partition

IN_DTYPE = np.float16

_EPS = 6.1e-5                  # smallest normal fp16; keeps Ln's input sane
_ACT_SET = "natural_log_exp_and_others"


def _preload_act_table(nc):
    """Pre-place one LoadActFuncSet for the set containing Square/Ln/Relu/Exp
    so Bacc.insert_act_table_loads doesn't thrash between greedy choices."""
    tables = list(get_activation_tables(nc.m.arch).keys())
    set_id = tables.index(_ACT_SET)
    inst = mybir.InstLoadActFuncSet(
        name=nc.get_next_instruction_name(), act_func_set_id=set_id, ins=[], outs=[]
    )
    return nc.scalar.add_instruction(inst)


def _build(W=3072, schedule=None, loop_reps=1, py_reps=1, f_split=1.0,
           planar_sq=True, preload_act=True, out_ring="act",
           bufs=(3, 8, 8, 6, 3), depth8=True):
    """`schedule`: optional explicit list of chunk widths (fp16 elems per
    partition, each a multiple of 6, summing to FPP). Default uniform W.
    `f_split`: fraction of the final out = m + c add done on GPSIMD (the
    rest runs on DVE). `bufs`: (io, d, sq, small) tile-pool depths.
    `loop_reps`: wrap the schedule in a hardware For_i loop (benchmarking
    steady-state HW time only).

    Emission is software-pipelined (modulo-scheduled): stage s of chunk i is
    emitted at tick i+s, so every engine's FIFO queue interleaves stages of
    DIFFERENT chunks and cross-engine semaphore waits are already satisfied
    when an instruction reaches the head of its queue.  Chunk-grouped
    emission measured 2x slower (lockstep: DVE idles during ACT's chain and
    vice versa, with later chunks stuck behind the stalled queue head)."""
    if schedule is None:
        assert W % 6 == 0 and FPP % W == 0
        schedule = [W] * (FPP // W)
    assert sum(schedule) == FPP and all(w % 6 == 0 for w in schedule)
    W = max(schedule)

    nc = bacc.Bacc("TRN2", target_bir_lowering=False, debug=False)

    x = nc.dram_tensor("x", [B_CORE, 3], F16, kind="ExternalInput")
    c = nc.dram_tensor("center", [B_CORE, 3], F16, kind="ExternalInput")
    o = nc.dram_tensor("out", [B_CORE, 3], F16, kind="ExternalOutput")

    xr = x.ap().rearrange("(p f) c -> p (f c)", p=P)
    cr = c.ap().rearrange("(p f) c -> p (f c)", p=P)
    orr = o.ap().rearrange("(p f) c -> p (f c)", p=P)

    # bufs = (x, center, d, sq, small): pool depth must cover each tile's
    # lifetime in pipeline ticks — ct is read by the FINAL add (stage 6), so
    # cp needs ~8 bufs or the whole pipeline throttles to its depth.
    b_x, b_c, b_d, b_sq, b_sm = bufs
    with tile.TileContext(nc) as tc, ExitStack() as ctx:
        if preload_act:
            _preload_act_table(nc)

        xp = ctx.enter_context(tc.tile_pool(name="xp", bufs=b_x))
        cp = ctx.enter_context(tc.tile_pool(name="cp", bufs=b_c))
        dp = ctx.enter_context(tc.tile_pool(name="dp", bufs=b_d))
        sqp = ctx.enter_context(tc.tile_pool(name="sqp", bufs=b_sq))
        sp = ctx.enter_context(tc.tile_pool(name="sp", bufs=b_sm))

        import contextlib
        loop_cm = tc.For_i(0, loop_reps, 1) if loop_reps > 1 else contextlib.nullcontext()
        with loop_cm:
            _emit_pipelined(nc, schedule * py_reps, W, xp, cp, dp, sqp, sp,
                            xr, cr, orr, f_split=f_split, planar_sq=planar_sq,
                            out_ring=out_ring, depth8=depth8)

    nc.compile()
    return nc


def _emit_pipelined(nc, schedule, W, xp, cp, dp, sqp, sp, xr, cr, orr,
                    f_split=1.0, planar_sq=True, out_ring="act", depth8=True):
    n = len(schedule)
    offs = [sum(schedule[:i]) % FPP for i in range(n)]
    st = [{} for _ in range(n)]          # per-chunk tile state
    rings = {"sp": nc.sync, "act": nc.scalar, "pool": nc.gpsimd}
    o_dma = rings[out_ring]

    def s0(i):                           # SP ring: inputs
        w, off = schedule[i], offs[i]
        xt = xp.tile([P, W], F16, name="xt", tag="xt")[:, :w]
        nc.sync.dma_start(xt[:, :], xr[:, off : off + w])
        ct = cp.tile([P, W], F16, name="ct", tag="ct")[:, :w]
        nc.sync.dma_start(ct[:, :], cr[:, off : off + w])
        st[i].update(xt=xt, ct=ct)

    def s1(i):                           # DVE: d = x - c
        w = schedule[i]
        dt = dp.tile([P, W], F16, name="dt", tag="dt")[:, :w]
        nc.vector.tensor_sub(dt[:, :], st[i]["xt"][:, :], st[i]["ct"][:, :])
        st[i]["dt"] = dt

    def s2(i):                           # ACT: squares, planar planes
        w, r = schedule[i], schedule[i] // 3
        sq = sqp.tile([P, W], F16, name="sq", tag="sq")[:, :w]
        d3 = st[i]["dt"].rearrange("p (r c) -> p r c", c=3)
        if planar_sq:
            sq_pl = sq.rearrange("p (c r) -> p r c", c=3)
            nc.scalar.activation(sq_pl[:, :, :], d3[:, :, :], AF.Square)
        else:
            for k in range(3):
                nc.scalar.activation(sq[:, k * r : (k + 1) * r], d3[:, :, k],
                                     AF.Square)
        st[i]["sq"] = sq

    def s3(i):                           # DVE: row sums (dense planes)
        r = schedule[i] // 3
        sq = st[i]["sq"]
        ta = sp.tile([P, W // 3], F16, name="ta", tag="ta")[:, :r]
        nc.vector.tensor_add(ta[:, :], sq[:, 0:r], sq[:, r : 2 * r])
        tb = sp.tile([P, W // 3], F16, name="tb", tag="tb")[:, :r]
        nc.vector.scalar_tensor_tensor(
            tb[:, :], ta[:, :], _EPS, sq[:, 2 * r : 3 * r], ALU.max, ALU.add
        )
        st[i].update(ta=ta, tb=tb)

    def s4(i):                           # ACT: scale = min(1, rsqrt(ss))
        ta, tb = st[i]["ta"], st[i]["tb"]
        r = schedule[i] // 3
        nc.scalar.activation(ta[:, :], tb[:, :], AF.Ln)
        nc.scalar.activation(tb[:, :], ta[:, :], AF.Relu)
        sc = sp.tile([P, W // 3], F16, name="sc", tag="sc")[:, :r]
        nc.scalar.activation(sc[:, :], tb[:, :], AF.Exp, scale=-0.5)
        st[i]["sc"] = sc

    def s5(i):                           # DVE: m_k = d_k * scale (into sq)
        sq, sc = st[i]["sq"], st[i]["sc"]
        d3 = st[i]["dt"].rearrange("p (r c) -> p r c", c=3)
        m3 = sq.rearrange("p (r c) -> p r c", c=3)
        for k in range(3):
            nc.vector.tensor_mul(m3[:, :, k], d3[:, :, k], sc[:, :])

    def s6(i):                           # Pool (+DVE head): out = m + c
        w = schedule[i]
        dt, sq, ct = st[i]["dt"], st[i]["sq"], st[i]["ct"]
        w1 = int(w * (1.0 - f_split)) // 6 * 6
        if w1 > 0:
            nc.vector.tensor_add(dt[:, :w1], sq[:, :w1], ct[:, :w1])
        if w1 < w:
            nc.gpsimd.tensor_add(dt[:, w1:], sq[:, w1:], ct[:, w1:])

    def s7(i):                           # out ring: result
        w, off = schedule[i], offs[i]
        o_dma.dma_start(orr[:, off : off + w], st[i]["dt"][:, :])
        st[i].clear()

    stages = [s0, s1, s2, s3, s4, s5, s6, s7]
    if not depth8:                       # merge compute into fewer ticks
        stages = [s0, s1, s2, lambda i: (s3(i), s4(i)),
                  lambda i: (s5(i), s6(i)), s7]
    depth = len(stages)
    for t in range(n + depth - 1):
        for s in range(depth - 1, -1, -1):
            i = t - s
            if 0 <= i < n:
                stages[s](i)


_NC = None

# 8 uniform 3072-wide chunks; with depth-8 modulo scheduling this keeps 8
# chunks in flight while fitting SBUF (dt pool alone needs 8 bufs).
_SCHEDULE = [3072] * 8


def _get_nc():
    global _NC
    if _NC is None:
        _NC = _build(schedule=_SCHEDULE)
    return _NC


def kernel(**inputs):
    x = np.asarray(inputs["x"], dtype=np.float32)
    center = np.asarray(inputs["center"], dtype=np.float32)
    assert x.shape == (B, 3) and center.shape == (B, 3)

    x16 = x.astype(np.float16)
    c16 = center.astype(np.float16)
    xs = x16.reshape(N_CORES, B_CORE, 3)
    cs = c16.reshape(N_CORES, B_CORE, 3)
    in_maps = [
        {"x": np.ascontiguousarray(xs[i]), "center": np.ascontiguousarray(cs[i])}
        for i in range(N_CORES)
    ]

    nc = _get_nc()
    res = run_bass_kernel_spmd(nc, in_maps, list(range(N_CORES)))
    out = np.concatenate([res.results[i]["out"] for i in range(N_CORES)], axis=0)
    return out.astype(np.float32)


if __name__ == "__main__":
    nc = _get_nc()
    print("build ok")


# revision 13
# speedup vs baseline: 1.4519x; 1.3218x over previous
"""Trainium2 Bass kernel for CircleProjectionLayer (ball projection, r=1).

out = center + d * min(1, 1/||d||),  d = x - center,  shapes [8388608, 3] f32.

Strategy vs the f32 baseline: the harness tolerance is rel_err < 2e-2 against
an output scale of ~4.6, while an end-to-end fp16 pipeline measures ~8e-4
relative error — so the whole kernel runs in fp16.  That halves HBM traffic
(the DMA roofline, this problem's target regime: 18 MiB/core vs 36) AND
doubles DVE throughput (fp16 tensor_tensor runs in 2x perf mode).

Sharding: pure data parallel — batch split 8 ways, one shard per NeuronCore.
Host casts f32 -> fp16 before upload and fp16 -> f32 after download (both
value-preserving to fp16 precision; the cast back is exact).

Per-core layout: the [1048576, 3] fp16 shard viewed flat as [128, 24576], so
each SBUF partition holds 8192 complete (x,y,z) rows contiguously; chunks of
W fp16 elements per partition stream through SBUF.

Engine split (all fp16), per chunk of R = W/3 rows:
  DVE   : d = x-c (dense, 2x); two dense row-sum adds over the sq planes;
          u = max(ss, 1) (tensor_scalar, 4x); v = 1/u (InstReciprocal);
          out = m + c (dense, 2x) — min(1, rsqrt(ss)) == rsqrt(max(ss,1)),
          an exact clamp with no eps and no inf path (recip input >= 1)
  ACT   : sq = Square(d) written PLANAR (one op, strided out) so the row
          sums read dense; scale = Sqrt(v).  Square and Sqrt coexist in the
          sqrt_and_others table set — the greedy table-load pass converges
          to it, unlike the Ln/Exp chain which thrashed LOAD 0 <-> LOAD 5
          at ~2.7 us per load, ~40 us/iteration of pure table loads.
  GPSIMD: m_k = d_k * scale (x3 strided muls; Pool's 2-input penalty is
          smallest here, and DVE runs strided muls at 1x anyway)
  DMA   : x-in + center-in on the SP HWDGE ring; out on the ACT HWDGE ring
          (SWDGE out-DMA costs the Pool sequencer ~4 us/chunk of descriptor
          generation, serialized against the Pool muls).

Emission is software-pipelined (modulo-scheduled) — see _build.
"""

import sys

sys.path.insert(0, "/opt/trn_rl_repo")

from contextlib import ExitStack

import numpy as np

import concourse.bass as bass
import concourse.tile as tile
from concourse import bacc, mybir
from concourse.bass_utils import run_bass_kernel_spmd
from concourse.hw_specs import get_activation_tables

F16 = mybir.dt.float16
AF = mybir.ActivationFunctionType
ALU = mybir.AluOpType

B = 8388608
N_CORES = 8
B_CORE = B // N_CORES          # 1048576 rows per core
P = 128
FPP = B_CORE * 3 // P          # 24576 fp16 elements per partition

IN_DTYPE = np.float16

_ACT_SET = "sqrt_and_others"   # contains both Square and Sqrt


def _preload_act_table(nc):
    """Pre-place one LoadActFuncSet for the set containing Square/Ln/Relu/Exp
    so Bacc.insert_act_table_loads doesn't thrash between greedy choices."""
    tables = list(get_activation_tables(nc.m.arch).keys())
    set_id = tables.index(_ACT_SET)
    inst = mybir.InstLoadActFuncSet(
        name=nc.get_next_instruction_name(), act_func_set_id=set_id, ins=[], outs=[]
    )
    return nc.scalar.add_instruction(inst)


def _build(W=3072, schedule=None, loop_reps=1, py_reps=1, f_split=1.0,
           planar_sq=True, preload_act=True, out_ring="act",
           bufs=(3, 8, 8, 6, 3), depth8=True):
    """`schedule`: optional explicit list of chunk widths (fp16 elems per
    partition, each a multiple of 6, summing to FPP). Default uniform W.
    `f_split`: fraction of the final out = m + c add done on GPSIMD (the
    rest runs on DVE). `bufs`: (io, d, sq, small) tile-pool depths.
    `loop_reps`: wrap the schedule in a hardware For_i loop (benchmarking
    steady-state HW time only).

    Emission is software-pipelined (modulo-scheduled): stage s of chunk i is
    emitted at tick i+s, so every engine's FIFO queue interleaves stages of
    DIFFERENT chunks and cross-engine semaphore waits are already satisfied
    when an instruction reaches the head of its queue.  Chunk-grouped
    emission measured 2x slower (lockstep: DVE idles during ACT's chain and
    vice versa, with later chunks stuck behind the stalled queue head)."""
    if schedule is None:
        assert W % 6 == 0 and FPP % W == 0
        schedule = [W] * (FPP // W)
    assert sum(schedule) == FPP and all(w % 6 == 0 for w in schedule)
    W = max(schedule)

    nc = bacc.Bacc("TRN2", target_bir_lowering=False, debug=False)

    x = nc.dram_tensor("x", [B_CORE, 3], F16, kind="ExternalInput")
    c = nc.dram_tensor("center", [B_CORE, 3], F16, kind="ExternalInput")
    o = nc.dram_tensor("out", [B_CORE, 3], F16, kind="ExternalOutput")

    xr = x.ap().rearrange("(p f) c -> p (f c)", p=P)
    cr = c.ap().rearrange("(p f) c -> p (f c)", p=P)
    orr = o.ap().rearrange("(p f) c -> p (f c)", p=P)

    # bufs = (x, center, d, sq, small): pool depth must cover each tile's
    # lifetime in pipeline ticks — ct is read by the FINAL add (stage 6), so
    # cp needs ~8 bufs or the whole pipeline throttles to its depth.
    b_x, b_c, b_d, b_sq, b_sm = bufs
    with tile.TileContext(nc) as tc, ExitStack() as ctx:
        if preload_act:
            _preload_act_table(nc)

        xp = ctx.enter_context(tc.tile_pool(name="xp", bufs=b_x))
        cp = ctx.enter_context(tc.tile_pool(name="cp", bufs=b_c))
        dp = ctx.enter_context(tc.tile_pool(name="dp", bufs=b_d))
        sqp = ctx.enter_context(tc.tile_pool(name="sqp", bufs=b_sq))
        sp = ctx.enter_context(tc.tile_pool(name="sp", bufs=b_sm))

        import contextlib
        loop_cm = tc.For_i(0, loop_reps, 1) if loop_reps > 1 else contextlib.nullcontext()
        with loop_cm:
            _emit_pipelined(nc, schedule * py_reps, W, xp, cp, dp, sqp, sp,
                            xr, cr, orr, f_split=f_split, planar_sq=planar_sq,
                            out_ring=out_ring, depth8=depth8)

    nc.compile()
    return nc


def _emit_pipelined(nc, schedule, W, xp, cp, dp, sqp, sp, xr, cr, orr,
                    f_split=1.0, planar_sq=True, out_ring="act", depth8=True):
    n = len(schedule)
    offs = [sum(schedule[:i]) % FPP for i in range(n)]
    st = [{} for _ in range(n)]          # per-chunk tile state
    rings = {"sp": nc.sync, "act": nc.scalar, "pool": nc.gpsimd}
    o_dma = rings[out_ring]

    def s0(i):                           # SP ring: inputs
        w, off = schedule[i], offs[i]
        xt = xp.tile([P, W], F16, name="xt", tag="xt")[:, :w]
        nc.sync.dma_start(xt[:, :], xr[:, off : off + w])
        ct = cp.tile([P, W], F16, name="ct", tag="ct")[:, :w]
        nc.sync.dma_start(ct[:, :], cr[:, off : off + w])
        st[i].update(xt=xt, ct=ct)

    def s1(i):                           # DVE: d = x - c
        w = schedule[i]
        dt = dp.tile([P, W], F16, name="dt", tag="dt")[:, :w]
        nc.vector.tensor_sub(dt[:, :], st[i]["xt"][:, :], st[i]["ct"][:, :])
        st[i]["dt"] = dt

    def s2(i):                           # ACT: squares, planar planes
        w, r = schedule[i], schedule[i] // 3
        sq = sqp.tile([P, W], F16, name="sq", tag="sq")[:, :w]
        d3 = st[i]["dt"].rearrange("p (r c) -> p r c", c=3)
        if planar_sq:
            sq_pl = sq.rearrange("p (c r) -> p r c", c=3)
            nc.scalar.activation(sq_pl[:, :, :], d3[:, :, :], AF.Square)
        else:
            for k in range(3):
                nc.scalar.activation(sq[:, k * r : (k + 1) * r], d3[:, :, k],
                                     AF.Square)
        st[i]["sq"] = sq

    def s3(i):                           # DVE: ss = max(|d|^2, 1); 1/ss
        r = schedule[i] // 3
        sq = st[i]["sq"]
        ta = sp.tile([P, W // 3], F16, name="ta", tag="ta")[:, :r]
        nc.vector.tensor_add(ta[:, :], sq[:, 0:r], sq[:, r : 2 * r])
        tb = sp.tile([P, W // 3], F16, name="tb", tag="tb")[:, :r]
        nc.vector.tensor_add(tb[:, :], ta[:, :], sq[:, 2 * r : 3 * r])
        # min(1, rsqrt(ss)) == rsqrt(max(ss, 1)) — exact clamp, no eps and
        # no inf/NaN path (reciprocal input is always >= 1).
        nc.vector.tensor_scalar_max(ta[:, :], tb[:, :], 1.0)
        with nc.allow_low_precision("harness tolerance 2e-2; fp16 throughout"):
            nc.vector.reciprocal(tb[:, :], ta[:, :])
        st[i].update(ta=ta, tb=tb)

    def s4(i):                           # ACT: scale = sqrt(1/max(ss,1))
        r = schedule[i] // 3
        sc = sp.tile([P, W // 3], F16, name="sc", tag="sc")[:, :r]
        nc.scalar.activation(sc[:, :], st[i]["tb"][:, :], AF.Sqrt)
        st[i]["sc"] = sc

    def s5(i):                           # Pool: m_k = d_k * scale (into sq)
        sq, sc = st[i]["sq"], st[i]["sc"]
        d3 = st[i]["dt"].rearrange("p (r c) -> p r c", c=3)
        m3 = sq.rearrange("p (r c) -> p r c", c=3)
        for k in range(3):
            nc.gpsimd.tensor_mul(m3[:, :, k], d3[:, :, k], sc[:, :])

    def s6(i):                           # DVE: out = m + c (dense, 2x)
        w = schedule[i]
        dt, sq, ct = st[i]["dt"], st[i]["sq"], st[i]["ct"]
        w1 = int(w * f_split) // 6 * 6   # f_split: DVE share of the add
        if w1 > 0:
            nc.vector.tensor_add(dt[:, :w1], sq[:, :w1], ct[:, :w1])
        if w1 < w:
            nc.gpsimd.tensor_add(dt[:, w1:], sq[:, w1:], ct[:, w1:])

    def s7(i):                           # out ring: result
        w, off = schedule[i], offs[i]
        o_dma.dma_start(orr[:, off : off + w], st[i]["dt"][:, :])
        st[i].clear()

    stages = [s0, s1, s2, s3, s4, s5, s6, s7]
    if not depth8:                       # merge compute into fewer ticks
        stages = [s0, s1, s2, lambda i: (s3(i), s4(i)),
                  lambda i: (s5(i), s6(i)), s7]
    depth = len(stages)
    for t in range(n + depth - 1):
        for s in range(depth - 1, -1, -1):
            i = t - s
            if 0 <= i < n:
                stages[s](i)


_NC = None

# 16 uniform 1536-wide chunks: the For_i loop drains all engines at each
# back-edge (iterations serialize), so the harness metric is single-shot
# latency — more, smaller chunks shrink the pipeline fill/drain overhead.
_SCHEDULE = [1536] * 16


def _get_nc():
    global _NC
    if _NC is None:
        _NC = _build(schedule=_SCHEDULE)
    return _NC


def kernel(**inputs):
    x = np.asarray(inputs["x"], dtype=np.float32)
    center = np.asarray(inputs["center"], dtype=np.float32)
    assert x.shape == (B, 3) and center.shape == (B, 3)

    x16 = x.astype(np.float16)
    c16 = center.astype(np.float16)
    xs = x16.reshape(N_CORES, B_CORE, 3)
    cs = c16.reshape(N_CORES, B_CORE, 3)
    in_maps = [
        {"x": np.ascontiguousarray(xs[i]), "center": np.ascontiguousarray(cs[i])}
        for i in range(N_CORES)
    ]

    nc = _get_nc()
    res = run_bass_kernel_spmd(nc, in_maps, list(range(N_CORES)))
    out = np.concatenate([res.results[i]["out"] for i in range(N_CORES)], axis=0)
    return out.astype(np.float32)


if __name__ == "__main__":
    nc = _get_nc()
    print("build ok")
